# revision 2
# baseline (speedup 1.0000x reference)
"""Multi-head attention block (B=8, N=1024, D=768, H=12 heads) on 8 trn2 NeuronCores.

Sharding: pure data-parallel over the batch dimension (one batch element per
core, weights replicated). No collectives needed.

Host-side prep (free, happens in numpy before upload): x is shipped
pre-transposed per core as xT [768, 1024] bf16 (no PE transposes needed),
W_qkv is split into wqk [768, 1536] (softmax scale folded into the q half)
and wv [768, 768], all weights in bf16 (halves DMA, full-rate matmuls).

Per-core kernel (Bass/Tile, all matmuls bf16 -> f32 PSUM):
  qT,kT = wqk.T-proj of xT    ([feature, token] layout, 12 x 128-row tiles)
  v = x @ W_v                 (natural layout, ones column FIRST per head)
  per head pair: S^T = kT.T-contract-qT tiles, P^T = exp(S^T)  (scale
            pre-folded; no max-sub: logits ~ N(0,1))
  out^T[1+d, n_q] = [1|v].T @ P^T  accumulated over k-tiles; row 0 = softmax
            denominator (partition 0 -> direct reciprocal_approx_fast);
            normalize via gpsimd partition_broadcast + tensor_mul (no DRAM
            round-trip)
  y = out.T @ W_proj + b_proj (outT-stationary matmuls)

Scheduling notes (v2):
  - xtw slab cols reordered to [wqk ft0 | xT | wv]; slab 0 is split into 3
    DMAs (and slab 1 into 2) so the first q/k matmuls fire as soon as the
    weight + first xT half land instead of waiting for the full 512KB slab.
  - wqk_rest arrives as one fat [128,768] DMA per feature tile (10 DMAs
    instead of 60 skinny ones), issued at step-list creation for prefetch.
  - output projection steps are fc-major: the first 384-col half's
    bias-add + y DMA overlaps the second half's matmuls.
  - filler steps are drained with an even per-kt quota across all remaining
    interleave slots (the old fixed pops=3 exhausted the list early and
    left the PE ActE-paced at ~250ns/kt).
  - last head pair runs 512/256/128/128-wide chunks so successively more
    token tiles' projections become interleavable; both 128-wide chunks
    normalize via the PE-broadcast path; only token tile 7's projection
    remains after the last attention chunk.
"""

import numpy as np

B, N, D = 8, 1024, 768
NH, HD = 12, 64
SCALE = HD ** -0.5  # 0.125
NT = N // 128       # 8 token tiles
NKT = D // 128      # 6 contraction tiles over D
NHP = NH // 2       # 6 head pairs

_STATE = {}


def _build():
    import concourse.bacc as bacc
    import concourse.bass as bass
    import concourse.mybir as mybir
    from concourse import tile

    f32 = mybir.dt.float32
    bf16 = mybir.dt.bfloat16
    EXP = mybir.ActivationFunctionType.Exp

    nc = bacc.Bacc(None, target_bir_lowering=False)
    # xtw row r = [q-ft0 weights row r (scale folded) | k-ft0 weights row r |
    # x^T row r | W_v row r] -- the whole startup-critical input stream
    # lands in a few fat, ordered DMAs per k-tile
    xtw = nc.dram_tensor("xtw", [D, 256 + N + D], bf16,
                         kind="ExternalInput")
    # remaining q/k feature tiles (q ft 1..5, then k ft 1..5), one
    # partition-major [128, 768] slab per ftile = one DMA each
    wqk_rest = nc.dram_tensor(
        "w_qk_rest", [10, 128, NKT * 128], bf16, kind="ExternalInput"
    )
    wproj = nc.dram_tensor(
        "w_proj", [128, NKT * D], bf16, kind="ExternalInput"
    )
    bproj = nc.dram_tensor("b_proj", [D], f32, kind="ExternalInput")
    y = nc.dram_tensor("y", [N, D], f32, kind="ExternalOutput")
    den_dram = nc.dram_tensor("den_scratch", [NH, N], f32)
    # global ftile index (1..5 = q, 7..11 = k) -> wqk_rest row
    REST_IDX = {1: 0, 2: 1, 3: 2, 4: 3, 5: 4, 7: 5, 8: 6, 9: 7, 10: 8, 11: 9}

    with tile.TileContext(nc) as tc:
        with (
            tc.tile_pool(name="const", bufs=1) as const,
            tc.tile_pool(name="big", bufs=1) as big,
            tc.tile_pool(name="ystage", bufs=8) as ystage,
        ):
            zb = const.tile([128, 1], f32)
            nc.vector.memset(zb[:], 0.0)
            onef = const.tile([128, 1], bf16)
            nc.vector.memset(onef[:], 1.0)
            ones_f = const.tile([128, HD], f32)
            nc.vector.memset(ones_f[:], 1.0)
            ones_t = const.tile([128, HD], mybir.dt.float32r)
            nc.vector.tensor_copy(ones_t[:], ones_f[:])
            bias_bc = const.tile([128, D], f32)

            # persistent activations
            xtw_sb = big.tile([128, NKT, 256 + N + D], bf16)
            wt_a1 = xtw_sb[:, :, 0:256]
            xT_sb = xtw_sb[:, :, 256:256 + N]
            wv_sb = xtw_sb[:, :, 256 + N:256 + N + D]
            qkT = big.tile([128, 2 * NHP, N], bf16)      # q ftiles 0..5, k 6..11
            vban = big.tile([128, NT, NH, HD + 1], bf16)  # v then ones col
            outT = big.tile([128, NHP, N], bf16)          # attention out, transposed
            wp_sb = big.tile([128, NKT, D], bf16)

            # W_proj + bias on the gpsimd queue (needed only from the
            # projection phase onwards)
            nc.gpsimd.dma_start(
                out=wp_sb[:].rearrange("p a b -> p (a b)"), in_=wproj[:, :]
            )
            nc.gpsimd.dma_start(
                out=bias_bc[:],
                in_=bass.AP(tensor=bproj, offset=0, ap=[[0, 128], [1, D]]),
            )

            # ones columns for the denominator trick (last index)
            nc.vector.tensor_copy(
                vban[:, :, :, HD:HD + 1].rearrange("p a b one -> p (a b one)"),
                onef[:, 0:1].to_broadcast((128, NT * NH)),
            )

            with (
                tc.tile_pool(name="wq_pool", bufs=3) as wq_pool,
                tc.tile_pool(name="pt_pool", bufs=4) as pt_pool,
                tc.tile_pool(name="s_ps", bufs=2, space="PSUM") as s_ps,
                tc.tile_pool(name="norm", bufs=2) as norm,
            ):
                ps_v_ctx = tc.tile_pool(name="ps_v", bufs=2, space="PSUM")
                ps_v = ps_v_ctx.__enter__()
                # ---- Phase A1: DMA-chased prologue ----
                # Per k-tile: wqk ft0 + xT + wv arrive, then q/k projection
                # for head pair 0 and v projection for token tiles 0/1
                # consume them. PE ramps with the DMA stream. Slab 0 is
                # split so the first matmuls' deps land early.
                psq01a = s_ps.tile([128, 1024], f32, tag="s", name="psq01a")
                psq01b = s_ps.tile([128, 1024], f32, tag="s", name="psq01b")
                psv_t0 = [
                    ps_v.tile([128, 384], f32, tag=f"vps{fc}", name=f"psv_t0_{fc}")
                    for fc in range(2)
                ]
                psv_t1 = [
                    ps_v.tile([128, 384], f32, tag=f"vps{fc}", name=f"psv_t1_{fc}")
                    for fc in range(2)
                ]
                for kt in range(NKT):
                    src = xtw[kt * 128:(kt + 1) * 128, :]
                    if kt == 0:
                        # wqk + xT[0:512] first: unblocks the very first
                        # q/k matmuls at ~37% of the slab
                        nc.sync.dma_start(xtw_sb[:, 0, 0:768], src[:, 0:768])
                        nc.sync.dma_start(
                            xtw_sb[:, 0, 768:1280], src[:, 768:1280]
                        )
                        nc.sync.dma_start(
                            xtw_sb[:, 0, 1280:2048], src[:, 1280:2048]
                        )
                    elif kt == 1:
                        nc.sync.dma_start(
                            xtw_sb[:, 1, 0:1280], src[:, 0:1280]
                        )
                        nc.sync.dma_start(
                            xtw_sb[:, 1, 1280:2048], src[:, 1280:2048]
                        )
                    else:
                        nc.sync.dma_start(xtw_sb[:, kt, :], src)
                    for qch in range(2):
                        nc.tensor.matmul(
                            psq01a[:, qch * 512:(qch + 1) * 512],
                            wt_a1[:, kt, 0:128],
                            xT_sb[:, kt, qch * 512:(qch + 1) * 512],
                            start=(kt == 0),
                            stop=(kt == NKT - 1),
                        )
                    for qch in range(2):
                        nc.tensor.matmul(
                            psq01b[:, qch * 512:(qch + 1) * 512],
                            wt_a1[:, kt, 128:256],
                            xT_sb[:, kt, qch * 512:(qch + 1) * 512],
                            start=(kt == 0),
                            stop=(kt == NKT - 1),
                        )
                    # v for token tiles 0/1 inline: wv[kt] arrives in the
                    # same slab stream as xT[kt], so this keeps the PE fed
                    # during the per-kt DMA chase
                    for tt, psvs in ((0, psv_t0), (1, psv_t1)):
                        for fc in range(2):
                            nc.tensor.matmul(
                                psvs[fc][:],
                                xT_sb[:, kt, tt * 128:(tt + 1) * 128],
                                wv_sb[:, kt, fc * 384:(fc + 1) * 384],
                                start=(kt == 0),
                                stop=(kt == NKT - 1),
                            )
                # evictions (ScalarE is idle here)
                for qch in range(2):
                    nc.scalar.copy(
                        qkT[:, 0, qch * 512:(qch + 1) * 512],
                        psq01a[:, qch * 512:(qch + 1) * 512],
                    )
                    nc.scalar.copy(
                        qkT[:, NHP, qch * 512:(qch + 1) * 512],
                        psq01b[:, qch * 512:(qch + 1) * 512],
                    )
                for tt, psvs in ((0, psv_t0), (1, psv_t1)):
                    for fc in range(2):
                        # split across engines so the banks free faster
                        eng = nc.scalar.copy if fc == 0 else \
                            nc.vector.tensor_copy
                        eng(
                            vban[:, tt, fc * 6:(fc + 1) * 6, 0:HD],
                            psvs[fc][:].rearrange("p (h d) -> p h d", h=6),
                        )

                # ---- Phase A2: v projection for token tiles 2..7 ----
                def emit_v(tt):
                    psv0 = ps_v.tile([128, 384], f32, tag="vps0", name="psv0")
                    psv1 = ps_v.tile([128, 384], f32, tag="vps1", name="psv1")
                    psvs = (psv0, psv1)
                    for kt in range(NKT):
                        for fc in range(2):
                            nc.tensor.matmul(
                                psvs[fc][:],
                                xT_sb[:, kt, tt * 128:(tt + 1) * 128],
                                wv_sb[:, kt, fc * 384:(fc + 1) * 384],
                                start=(kt == 0),
                                stop=(kt == NKT - 1),
                            )
                    for fc in range(2):
                        eng = nc.scalar.copy if fc == 0 else \
                            nc.vector.tensor_copy
                        eng(
                            vban[:, tt, fc * 6:(fc + 1) * 6, 0:HD],
                            psvs[fc][:].rearrange("p (h d) -> p h d", h=6),
                        )

                for tt in range(2, NT):
                    emit_v(tt)
                ps_v_ctx.__exit__(None, None, None)

                acc_ps_ctx = tc.tile_pool(name="acc_ps", bufs=1, space="PSUM")
                acc_ps = acc_ps_ctx.__enter__()
                qk_ps_ctx = tc.tile_pool(name="qk_ps", bufs=1, space="PSUM")
                qk_ps = qk_ps_ctx.__enter__()

                # ---- Phase B: attention with q/k projection interleaved ----
                def qk_steps_for(ft):
                    """Generator of closures; each emits one PE step of the
                    qT/kT projection for feature tile ft (2 psum halves).
                    All 6 weight k-tiles for the ftile arrive as one fat
                    DMA, issued at step-list creation time (prefetch)."""
                    psq0 = qk_ps.tile([128, 512], f32, tag="psq0", name="psq0")
                    psq1 = qk_ps.tile([128, 512], f32, tag="psq1", name="psq1")
                    psqs = (psq0, psq1)
                    wt = wq_pool.tile([128, NKT * 128], bf16, tag="wt",
                                      name="wt")
                    nc.sync.dma_start(wt[:], wqk_rest[REST_IDX[ft]][:, :])

                    def mk_mm(kt):
                        def emit():
                            for qch in range(2):
                                nc.tensor.matmul(
                                    psqs[qch][:],
                                    wt[:, kt * 128:(kt + 1) * 128],
                                    xT_sb[:, kt, qch * 512:(qch + 1) * 512],
                                    start=(kt == 0),
                                    stop=(kt == NKT - 1),
                                )
                        return emit

                    def mk_evict():
                        def emit():
                            for qch in range(2):
                                nc.vector.tensor_copy(
                                    qkT[:, ft, qch * 512:(qch + 1) * 512],
                                    psqs[qch][:],
                                )
                        return emit

                    return [mk_mm(kt) for kt in range(NKT)] + [mk_evict()]

                def proj_steps_for(tt):
                    """Output projection of token tile tt as interleavable
                    fc-major steps (borrows the idle qk PSUM slots). The
                    first 384-col half's bias-add + y DMA overlaps the
                    second half's matmuls."""
                    psys = {
                        0: qk_ps.tile([128, 512], f32, tag="psq0",
                                      name="psy0"),
                        1: qk_ps.tile([128, 512], f32, tag="psq1",
                                      name="psy1"),
                    }

                    def mk_mm(fc, j0):
                        def emit():
                            for j in (j0, j0 + 1, j0 + 2):
                                nc.tensor.matmul(
                                    psys[fc][:, 0:384],
                                    outT[:, j, tt * 128:(tt + 1) * 128],
                                    wp_sb[:, j, fc * 384:(fc + 1) * 384],
                                    start=(j == 0),
                                    stop=(j == NHP - 1),
                                )
                        return emit

                    def mk_evict(fc):
                        def emit():
                            yst = ystage.tile([128, 384], f32, tag="yst",
                                              name="yst")
                            nc.vector.tensor_add(
                                yst[:], psys[fc][:, 0:384],
                                bias_bc[:, fc * 384:(fc + 1) * 384],
                            )
                            nc.sync.dma_start(
                                y[tt * 128:(tt + 1) * 128,
                                  fc * 384:(fc + 1) * 384],
                                yst[:],
                            )
                        return emit

                    return [
                        mk_mm(0, 0), mk_mm(0, 3), mk_evict(0),
                        mk_mm(1, 0), mk_mm(1, 3), mk_evict(1),
                    ]

                def do_chunk(hp, c0, cw, pending, defer=0, pe_norm=False):
                    """One attention chunk: q columns [c0, c0+cw) for head
                    pair hp; pending steps drained at an even per-kt quota
                    sized for the remaining NT-kt slots here plus `defer`
                    future slots in later chunks."""
                    acc0 = acc_ps.tile([HD + 1, 512], f32, tag="acc0",
                                       name="acc0")
                    acc1 = acc_ps.tile([HD + 1, 512], f32, tag="acc1",
                                       name="acc1")
                    accs = (acc0, acc1)
                    pts = []

                    def emit_av(kt):
                        pt = pts[kt]
                        for h in range(2):
                            nc.tensor.matmul(
                                accs[h][:, 0:cw],
                                vban[:, kt, hp * 2 + h, :],
                                pt[:, h * cw:(h + 1) * cw],
                                start=(kt == 0),
                                stop=(kt == NT - 1),
                            )

                    for kt in range(NT):
                        ssum = s_ps.tile([128, 1024], f32, tag="s",
                                         name="ssum")
                        for h in range(2):
                            ksl = qkT[h * 64:(h + 1) * 64, NHP + hp,
                                      kt * 128:(kt + 1) * 128]
                            qsl = qkT[h * 64:(h + 1) * 64, hp, c0:c0 + cw]
                            # h slices stay bank-aligned (offset h*512) so
                            # the two matmul groups land in separate banks
                            nc.tensor.matmul(
                                ssum[:, h * 512:h * 512 + cw],
                                ksl,
                                qsl,
                                start=True,
                                stop=True,
                            )
                        pt = pt_pool.tile([128, 1024], bf16, tag="pt",
                                          name="pt")
                        ssv = ssum[:].rearrange(
                            "p (b c) -> p b c", b=2)[:, :, 0:cw]
                        ptv = pt[:, 0:2 * cw].rearrange(
                            "p (b c) -> p b c", b=2)
                        nc.scalar.activation(
                            ptv, ssv, EXP, bias=zb[:], scale=1.0
                        )
                        pts.append(pt)
                        if kt >= 1:
                            emit_av(kt - 1)
                        slots = NT - kt + defer
                        quota = (len(pending) + slots - 1) // slots \
                            if pending else 0
                        for _ in range(quota):
                            if pending:
                                pending.pop(0)()
                    emit_av(NT - 1)

                    # evict accumulators to SBUF immediately (single PSUM
                    # reader -> acc banks free early), then normalize
                    adt = mybir.dt.float32r if pe_norm else f32
                    asb0 = norm.tile([HD + 1, 512], adt, tag="asb0",
                                     name="asb0")
                    asb1 = norm.tile([HD + 1, 512], adt, tag="asb1",
                                     name="asb1")
                    asbs = (asb0, asb1)
                    nc.vector.tensor_copy(asb0[:, 0:cw], acc0[:, 0:cw])
                    nc.vector.tensor_copy(asb1[:, 0:cw], acc1[:, 0:cw])
                    if pe_norm:
                        # tail-only: broadcast the denominator row via a
                        # K=1 ones matmul into the just-freed acc bank,
                        # then reciprocal + multiply on DVE. No DMA
                        # round-trip latency on the critical tail.
                        for h in range(2):
                            rbp = acc_ps.tile([128, 512], f32,
                                              tag=f"acc{h}", name=f"rbp{h}")
                            nc.tensor.matmul(
                                rbp[0:HD, 0:cw],
                                ones_t[HD:HD + 1, 0:HD],
                                asbs[h][HD:HD + 1, 0:cw],
                                start=True,
                                stop=True,
                            )
                            rbr = norm.tile([HD, 512], f32, tag=f"rb{h}",
                                            name=f"rb{h}")
                            nc.vector.reciprocal_approx_fast(
                                rbr[:, 0:cw], rbp[0:HD, 0:cw]
                            )
                            nc.vector.tensor_mul(
                                outT[h * 64:(h + 1) * 64, hp, c0:c0 + cw],
                                asbs[h][0:HD, 0:cw],
                                rbr[:, 0:cw],
                            )
                        return
                    # denominator rows -> partition 0 via SBUF->SBUF DMA
                    # (custom DVE recip is broken at base_partition != 0).
                    # DMAs ride the idle gpsimd queue, off the sync queue.
                    dd = norm.tile([1, N], f32, tag="dd", name="dd")
                    for h in range(2):
                        nc.gpsimd.dma_start(
                            out=dd[0:1, h * cw:(h + 1) * cw],
                            in_=asbs[h][HD:HD + 1, 0:cw],
                        )
                    rr = norm.tile([1, N], f32, tag="rr", name="rr")
                    nc.vector.reciprocal_approx_fast(
                        rr[0:1, 0:2 * cw], dd[0:1, 0:2 * cw]
                    )
                    for h in range(2):
                        nc.gpsimd.dma_start(
                            out=den_dram[2 * hp + h][None, c0:c0 + cw],
                            in_=rr[0:1, h * cw:(h + 1) * cw],
                        )
                    # partition-broadcast via DRAM round-trip
                    # (zero-stride partition APs need a DRAM source)
                    for h in range(2):
                        rb = norm.tile([HD, 512], f32, tag=f"rb{h}",
                                       name=f"rb{h}")
                        bcast_ap = bass.AP(
                            tensor=den_dram,
                            offset=(hp * 2 + h) * N + c0,
                            ap=[[0, HD], [1, cw]],
                        )
                        nc.gpsimd.dma_start(out=rb[:, 0:cw], in_=bcast_ap)
                        nc.vector.tensor_mul(
                            outT[h * 64:(h + 1) * 64, hp, c0:c0 + cw],
                            asbs[h][0:HD, 0:cw],
                            rb[:, 0:cw],
                        )

                for hp in range(NHP - 1):
                    # qk steps for the next head pair, spread across this
                    # head pair's 16 attention chunks
                    pending = qk_steps_for(hp + 1) + qk_steps_for(
                        NHP + hp + 1
                    )
                    do_chunk(hp, 0, 512, pending, defer=NT)
                    do_chunk(hp, 512, 512, pending)
                    for step in pending:
                        step()
                # last head pair: successively narrower chunks so more
                # token tiles' projections become interleavable; only tt7
                # remains after the final attention chunk
                do_chunk(NHP - 1, 0, 512, [])
                pending = []
                for tt_ in range(4):
                    pending += proj_steps_for(tt_)
                do_chunk(NHP - 1, 512, 256, pending, defer=2 * NT)
                pending += proj_steps_for(4) + proj_steps_for(5)
                do_chunk(NHP - 1, 768, 128, pending, defer=NT, pe_norm=True)
                pending += proj_steps_for(6)
                do_chunk(NHP - 1, 896, 128, pending, pe_norm=True)
                for step in pending:
                    step()
                # tt7: fc-major, so only the second half's bias-add + DMA
                # trails the last matmul
                for step in proj_steps_for(7):
                    step()
                qk_ps_ctx.__exit__(None, None, None)
                acc_ps_ctx.__exit__(None, None, None)

    nc.compile()
    return nc


def make_in_maps(inputs):
    """Host-side prep: per-core sharding + transpose + bf16 cast (numpy)."""
    import ml_dtypes

    bf16 = ml_dtypes.bfloat16
    x = np.asarray(inputs["x"], dtype=np.float32)
    wqkv = np.asarray(inputs["W_qkv"], dtype=np.float32)
    wproj = np.asarray(inputs["W_proj"], dtype=np.float32)
    bproj = np.asarray(inputs["b_proj"], dtype=np.float32)

    wq = wqkv[:, 0:D] * SCALE
    wk = wqkv[:, D:2 * D]

    def pack(w):
        # [768, C] -> partition-major [128, 6*C]: row p holds the weight
        # rows {kt*128+p} for kt=0..5, concatenated (contiguous DMA image)
        c = w.shape[1]
        return w.reshape(NKT, 128, c).transpose(1, 0, 2).reshape(128, NKT * c)

    rest = [pack(wq[:, ft * 128:(ft + 1) * 128]) for ft in range(1, 6)]
    rest += [pack(wk[:, ft * 128:(ft + 1) * 128]) for ft in range(1, 6)]
    wqk_rest = np.ascontiguousarray(np.stack(rest, axis=0)).astype(bf16)
    wproj_b = np.ascontiguousarray(pack(wproj)).astype(bf16)
    bproj_c = np.ascontiguousarray(bproj)
    # startup stream: [q ft0 (scaled) | k ft0 | x^T | W_v] row-aligned
    wblock = np.concatenate([wq[:, 0:128], wk[:, 0:128]], axis=1)
    wv = wqkv[:, 2 * D:3 * D]
    return [
        {
            "xtw": np.ascontiguousarray(
                np.concatenate([wblock, x[b].T, wv], axis=1)
            ).astype(bf16),
            "w_qk_rest": wqk_rest,
            "w_proj": wproj_b,
            "b_proj": bproj_c,
        }
        for b in range(B)
    ]


def kernel(**inputs) -> np.ndarray:
    from concourse.bass_utils import run_bass_kernel_spmd

    if "nc" not in _STATE:
        _STATE["nc"] = _build()
    nc = _STATE["nc"]

    in_maps = make_in_maps(inputs)
    res = run_bass_kernel_spmd(nc, in_maps, list(range(B)))
    out = np.stack([res.results[b]["y"] for b in range(B)], axis=0)
    return out.astype(np.float32)
